# revision 6
# baseline (speedup 1.0000x reference)
"""Trainium2 Bass kernel for nn_GammaFFNNLayer (RK4 step of dgamma/dt = f(eps,gamma)*(eps-gamma),
f = MLP 2->32(tanh)->32(tanh)->1(softplus)).

Data-parallel over 8 cores. Per core: 131072 samples laid out as a dense
[128 partitions x 1024 cols] tile. The MLP runs as 32 "blocks" (4 samples-partitions
per block -> 4 groups x 32 hidden = 128 partitions), using host-precomputed
scatter/gather weight matrices so layer-1 reads and layer-3 writes the dense
sample layout directly via matmuls (PSUM accumulation builds the dense f tile).
All matmuls run as float32r (full-rate fp32 path on the PE).
"""

import numpy as np

import concourse.bass as bass
import concourse.mybir as mybir
import concourse.tile as tile
from concourse import bacc
from concourse.bass_utils import run_bass_kernel_spmd

N_CORES = 8
B_FULL = 1 << 20
PER_CORE = B_FULL // N_CORES  # 131072
P = 128
H = 32
NBLK = 32  # sample-partition blocks (4 partitions each)
NCOL = PER_CORE // P  # 1024
F32 = mybir.dt.float32
F32R = mybir.dt.float32r

MULT = mybir.AluOpType.mult
ADD = mybir.AluOpType.add


def build_bass(ncol=NCOL, use_f32r=True):
    """Build the single-core Bass/Tile program (SPMD across cores)."""
    nc = bacc.Bacc(None, target_bir_lowering=False)

    x_d = nc.dram_tensor("x", [P, 4 * ncol], F32, kind="ExternalInput")
    g_d = nc.dram_tensor("g", [P, ncol], F32R, kind="ExternalInput")
    wt_d = nc.dram_tensor("wt", [P, NBLK * P], F32R, kind="ExternalInput")
    ws_d = nc.dram_tensor("ws", [P, NBLK * P], F32R, kind="ExternalInput")
    wu_d = nc.dram_tensor("wu", [P, NBLK * P], F32R, kind="ExternalInput")
    w2_d = nc.dram_tensor("w2", [P, P], F32R, kind="ExternalInput")
    bias_d = nc.dram_tensor("bias", [P, 4], F32, kind="ExternalInput")
    o_d = nc.dram_tensor("o", [P, ncol], F32, kind="ExternalOutput")

    MMF = min(512, ncol)  # matmul free-dim chunk (one PSUM bank, fp32)
    nh = ncol // MMF
    Tanh = mybir.ActivationFunctionType.Tanh
    Exp = mybir.ActivationFunctionType.Exp
    Ln = mybir.ActivationFunctionType.Ln

    if use_f32r:
        def r(ap):
            return ap.bitcast(F32R)
    else:
        def r(ap):
            return ap

    with tile.TileContext(nc) as tc:
        with (
            tc.tile_pool(name="consts", bufs=1) as consts,
            tc.tile_pool(name="data", bufs=1) as data,
            tc.tile_pool(name="work", bufs=2) as work,
            tc.tile_pool(name="keep", bufs=1) as keep,
            tc.tile_pool(name="h", bufs=3) as hpool,
            tc.tile_pool(name="ph1", bufs=2, space="PSUM") as ph1p,
            tc.tile_pool(name="ph2", bufs=1, space="PSUM") as ph2p,
            tc.tile_pool(name="pf", bufs=1, space="PSUM") as pfp,
        ):
            wt = consts.tile([P, NBLK * P], F32R)
            nc.sync.dma_start(wt[:], wt_d[:])
            ws = consts.tile([P, NBLK * P], F32R)
            nc.sync.dma_start(ws[:], ws_d[:])
            wu = consts.tile([P, NBLK * P], F32R)
            nc.sync.dma_start(wu[:], wu_d[:])
            w2 = consts.tile([P, P], F32R)
            nc.sync.dma_start(w2[:], w2_d[:])
            biast = consts.tile([P, 4], F32)
            nc.sync.dma_start(biast[:], bias_d[:])
            xr = data.tile([P, 4 * ncol], F32)
            nc.sync.dma_start(xr[:], x_d[:])
            gam0 = data.tile([P, ncol], F32R)
            nc.sync.dma_start(gam0[:], g_d[:])

            xv = xr.rearrange("p (n c) -> p n c", c=4)
            eps1 = data.tile([P, ncol], F32R)
            nc.vector.tensor_copy(eps1[:], xv[:, :, 0])
            epsh = data.tile([P, ncol], F32R)
            nc.vector.tensor_copy(epsh[:], xv[:, :, 1])
            eps2 = data.tile([P, ncol], F32R)
            nc.vector.tensor_copy(eps2[:], xv[:, :, 2])
            dtf = data.tile([P, ncol], F32)
            nc.vector.tensor_copy(dtf[:], xv[:, :, 3])
            dth = data.tile([P, ncol], F32)
            nc.vector.tensor_scalar_mul(dth[:], xv[:, :, 3], 0.5)

            b1t = biast[:, 0:1]
            b2t = biast[:, 1:2]
            b3t = biast[:, 2:3]

            eps_by_stage = [eps1, epsh, epsh, eps2]
            dt_by_stage = [dth, dth, dtf, dtf]

            gcur = gam0
            ms = []
            for s in range(4):
                eps_s = eps_by_stage[s]
                dt_s = dt_by_stage[s]
                # pre_s = (eps_s - gamma_s) * dt_scaled   (off the critical path)
                tpre = work.tile([P, ncol], F32, tag="tpre")
                nc.vector.scalar_tensor_tensor(
                    tpre[:], gcur.bitcast(F32)[:], -1.0, eps_s.bitcast(F32)[:], MULT, ADD
                )
                pre = work.tile([P, ncol], F32, tag="pre")
                nc.vector.tensor_mul(pre[:], tpre[:], dt_s[:])

                fpre = pfp.tile([P, ncol], F32, tag="fpre")
                for b in range(NBLK):
                    wtb = wt[:, b * P:(b + 1) * P]
                    wsb = ws[:, b * P:(b + 1) * P]
                    wub = wu[:, b * P:(b + 1) * P]
                    ph1 = ph1p.tile([P, ncol], F32, tag="ph1")
                    for hh in range(nh):
                        sl = slice(hh * MMF, (hh + 1) * MMF)
                        nc.tensor.matmul(
                            ph1[:, sl], wtb, eps_s[:, sl],
                            start=True, stop=False,
                        )
                        nc.tensor.matmul(
                            ph1[:, sl], wsb, gcur[:, sl],
                            start=False, stop=True,
                        )
                    h1 = hpool.tile([P, ncol], F32R, tag="h1")
                    nc.scalar.activation(h1[:], ph1[:], Tanh, bias=b1t, scale=1.0)
                    ph2 = ph2p.tile([P, ncol], F32, tag="ph2")
                    for hh in range(nh):
                        sl = slice(hh * MMF, (hh + 1) * MMF)
                        nc.tensor.matmul(
                            ph2[:, sl], w2[:], h1[:, sl],
                            start=True, stop=True,
                        )
                    h2 = hpool.tile([P, ncol], F32R, tag="h2")
                    nc.scalar.activation(h2[:], ph2[:], Tanh, bias=b2t, scale=1.0)
                    for hh in range(nh):
                        sl = slice(hh * MMF, (hh + 1) * MMF)
                        nc.tensor.matmul(
                            fpre[:, sl], wub, h2[:, sl],
                            start=(b == 0), stop=(b == NBLK - 1),
                        )
                # softplus(z + b3) = ln(1 + exp(z + b3)); Exp shares the tanh
                # table set so only Ln costs a table switch.
                ez = work.tile([P, ncol], F32, tag="ez")
                nc.scalar.activation(ez[:], fpre[:], Exp, bias=b3t, scale=1.0)
                ez1 = work.tile([P, ncol], F32, tag="ez1")
                nc.vector.tensor_scalar_add(ez1[:], ez[:], 1.0)
                fs = work.tile([P, ncol], F32, tag="f")
                nc.scalar.activation(fs[:], ez1[:], Ln, bias=0.0, scale=1.0)
                m = keep.tile([P, ncol], F32, tag=f"m{s}")
                nc.vector.tensor_mul(m[:], fs[:], pre[:])
                ms.append(m)
                if s < 3:
                    gn = work.tile([P, ncol], F32R, tag="g")
                    nc.vector.tensor_add(gn[:], gam0.bitcast(F32)[:], m[:])
                    gcur = gn

            # gamma_new = gamma + (2*m0 + 4*m1 + 2*m2 + m3) / 6
            x1 = keep.tile([P, ncol], F32, tag="x1")
            nc.vector.scalar_tensor_tensor(x1[:], ms[1][:], 2.0, ms[0][:], MULT, ADD)
            x2 = keep.tile([P, ncol], F32, tag="x2")
            nc.vector.tensor_add(x2[:], x1[:], ms[2][:])
            x3 = keep.tile([P, ncol], F32, tag="x3")
            nc.vector.scalar_tensor_tensor(x3[:], x2[:], 2.0, ms[3][:], MULT, ADD)
            gout = keep.tile([P, ncol], F32, tag="gout")
            nc.vector.scalar_tensor_tensor(gout[:], x3[:], 1.0 / 6.0, gam0.bitcast(F32)[:], MULT, ADD)
            nc.sync.dma_start(o_d[:], gout[:])

    return nc


def make_weight_images(W1, b1, W2, b2, W3, b3):
    """Host-side constant weight matrices in SBUF image layout [128, ...]."""
    W1 = np.asarray(W1, np.float32)
    W2 = np.asarray(W2, np.float32)
    W3 = np.asarray(W3, np.float32)
    T_all = np.zeros((NBLK, P, P), np.float32)  # lhsT: eps gather -> h1 rows
    S_all = np.zeros((NBLK, P, P), np.float32)  # lhsT: gamma gather -> h1 rows
    U_all = np.zeros((NBLK, P, P), np.float32)  # lhsT: h2 -> dense f scatter
    for b in range(NBLK):
        for i in range(4):
            p = 4 * b + i
            T_all[b, p, 32 * i:32 * i + 32] = W1[0]
            S_all[b, p, 32 * i:32 * i + 32] = W1[1]
            U_all[b, 32 * i:32 * i + 32, p] = W3[:, 0]
    W2bd = np.zeros((P, P), np.float32)
    for i in range(4):
        W2bd[32 * i:32 * i + 32, 32 * i:32 * i + 32] = W2
    wt_img = np.ascontiguousarray(T_all.transpose(1, 0, 2).reshape(P, NBLK * P))
    ws_img = np.ascontiguousarray(S_all.transpose(1, 0, 2).reshape(P, NBLK * P))
    wu_img = np.ascontiguousarray(U_all.transpose(1, 0, 2).reshape(P, NBLK * P))
    bias_img = np.zeros((P, 4), np.float32)
    bias_img[:, 0] = np.tile(np.asarray(b1, np.float32), 4)
    bias_img[:, 1] = np.tile(np.asarray(b2, np.float32), 4)
    bias_img[:, 2] = float(np.asarray(b3, np.float32).reshape(-1)[0])
    return wt_img, ws_img, wu_img, W2bd, bias_img


_NC_CACHE = {}


def _get_nc():
    if "nc" not in _NC_CACHE:
        nc = build_bass(NCOL, use_f32r=True)
        nc.compile()  # bacc passes: register alloc, act-table loads, DCE
        _NC_CACHE["nc"] = nc
    return _NC_CACHE["nc"]


def kernel(**inputs):
    x = np.ascontiguousarray(np.asarray(inputs["inputs"], np.float32))
    gam = np.ascontiguousarray(np.asarray(inputs["gamma"], np.float32))
    wt_img, ws_img, wu_img, W2bd, bias_img = make_weight_images(
        inputs["W1"], inputs["b1"], inputs["W2"], inputs["b2"],
        inputs["W3"], inputs["b3"],
    )
    nc = _get_nc()
    in_maps = []
    for c in range(N_CORES):
        sh = slice(c * PER_CORE, (c + 1) * PER_CORE)
        in_maps.append({
            "x": x[sh].reshape(P, 4 * NCOL),
            "g": gam[sh].reshape(P, NCOL),
            "wt": wt_img, "ws": ws_img, "wu": wu_img,
            "w2": W2bd, "bias": bias_img,
        })
    res = run_bass_kernel_spmd(nc, in_maps, list(range(N_CORES)))
    out = np.concatenate(
        [res.results[c]["o"].reshape(PER_CORE, 1) for c in range(N_CORES)], axis=0
    )
    return out.astype(np.float32)
